# revision 14
# baseline (speedup 1.0000x reference)
"""Real spherical harmonics Y_l^m (l=0..7) over N=2M points on 8 TRN2 NeuronCores.

Design (fp16 compute, ~2x DVE rate, ~1.5e-3 rel err vs f32 reference):
  - Data-parallel over points: each core gets NSH=250112 points (global input
    padded from 2,000,000 to 2,000,896).
  - Per core, tiles of [128 partitions, F free]; point id = tile_off + p*F + f.
  - Output stored column-major in DRAM ([64 rows, NSH] fp16, m-major row
    order) so every engine write and DMA is contiguous; host converts to f32
    and regroups per-l.
  - Scaled Legendre recurrences: Pt(l,m) = P(l,m)/gamma(l,m) with gamma chosen
    so  Pt(m,m)=s*Pt(m-1,m-1), Pt(m+1,m)=x*Pt(m,m),
        Pt(l,m)=c(l,m)*x*Pt(l-1,m) - Pt(l-2,m)
  - sin/cos(m*phi) on ScalarE: fp32 round-to-nearest trick (+1.5*2^23) for
    range reduction, then Sin activations (table valid on [-3.4, 3.4]).
  - Columns: Phat(l,m) = K(l,m)*Pt(l,m) (tensor_scalar, 4x fp16), written
    contiguously per m; then ONE wide tensor_tensor per (m, sin/cos) using a
    stride-0 broadcast AP of the trig tile (2x fp16).
  - Work split across VectorE / GpSimd / ScalarE; DMA via HWDGE (nc.sync).
"""
import sys, math, dataclasses

if '/opt/trn_rl_repo' not in sys.path:
    sys.path.insert(0, '/opt/trn_rl_repo')

import numpy as np
import concourse.bass as bass
import concourse.tile as tile
from concourse import bacc, mybir
from concourse.bass_utils import run_bass_kernel_spmd

AFT = mybir.ActivationFunctionType
ALU = mybir.AluOpType
F32 = mybir.dt.float32
F16 = mybir.dt.float16

N_FULL = 2_000_000
NCORES = 8
LMAX = 7
NCOL = 64
NSH = 250_112                           # 128 * 1954
N_PAD = NSH * NCORES
TILE_FS = [652, 652, 650]
PI = math.pi
RND_C = float(3 * 2 ** 22)
GP_COMPUTE = False                      # GpSimd contends with VectorE SBUF ports
TS_ENGINE = 'scalar'                    # Phat scaling on ScalarE (Copy w/ scale)
CHUNK_BUFS = 3
PHAT_BUFS = 2
TRIG_BUFS = 2

# DRAM row order (m-major): rows 0..7 = (l,0) l=0..7; then per m: sin rows
# (l=m..7), cos rows (l=m..7).
ROW0 = {}
_r = 8
ROWS_SIN, ROWS_COS = {}, {}
for _m in range(1, 8):
    ROWS_SIN[_m] = _r; _r += 8 - _m
    ROWS_COS[_m] = _r; _r += 8 - _m
assert _r == 64

# PERM[c_lmajor] = dram row for global column c = l*l + l + m
PERM = np.zeros(64, np.int64)
for _l in range(8):
    for _mm in range(-_l, _l + 1):
        _c = _l * _l + _l + _mm
        if _mm == 0:
            PERM[_c] = _l
        elif _mm < 0:
            PERM[_c] = ROWS_SIN[-_mm] + (_l - (-_mm))
        else:
            PERM[_c] = ROWS_COS[_mm] + (_l - _mm)


def _constants():
    g = {(0, 0): 1.0}
    c = {}
    for m in range(1, LMAX + 1):
        g[(m, m)] = g[(m - 1, m - 1)] * (-(2 * m - 1))
    for m in range(0, LMAX):
        g[(m + 1, m)] = g[(m, m)] * (2 * m + 1)
    for m in range(0, LMAX + 1):
        for l in range(m + 2, LMAX + 1):
            g[(l, m)] = (l + m - 1) * g[(l - 2, m)] / (l - m)
            c[(l, m)] = (2 * l - 1) * g[(l - 1, m)] / ((l - m) * g[(l, m)])
    kt = {}
    for l in range(LMAX + 1):
        for m in range(0, l + 1):
            kn = math.sqrt((2 * l + 1) / (4 * math.pi)
                           * math.factorial(l - m) / math.factorial(l + m))
            kt[(l, m)] = kn * g[(l, m)] * (math.sqrt(2.0) if m > 0 else 1.0)
    return c, kt


CREC, KT = _constants()


def _rep(ap, r):
    """Broadcast a [128, F] AP to [128, r, F] via a stride-0 middle dim."""
    return dataclasses.replace(ap, ap=[ap.ap[0], [0, r], ap.ap[1]])


def _build():
    nc = bacc.Bacc("TRN2", target_bir_lowering=False, debug=False,
                   num_devices=NCORES)

    def reg_const(value):
        t = nc.alloc_sbuf_tensor(f"cst{value}", [128, 1], F32)
        nc.gpsimd.memset(t.ap(), value)
        nc.const_aps.aps[(F32, value)] = t.ap()

    reg_const(PI / 2)
    reg_const(RND_C)
    reg_const(-RND_C)
    nc.all_engine_barrier()

    ct_d = nc.dram_tensor("ct", [NSH], F32, kind="ExternalInput")
    ph_d = nc.dram_tensor("phi", [NSH], F32, kind="ExternalInput")
    out_d = nc.dram_tensor("out", [NCOL, NSH], F16, kind="ExternalOutput")

    with tile.TileContext(nc) as tc:
        with (tc.tile_pool(name="io", bufs=2) as io,
              tc.tile_pool(name="sca", bufs=2) as sca,
              tc.tile_pool(name="trig", bufs=TRIG_BUFS) as trig,
              tc.tile_pool(name="leg", bufs=2) as leg,
              tc.tile_pool(name="tmp", bufs=4) as tmppool,
              tc.tile_pool(name="phat", bufs=PHAT_BUFS) as phatpool,
              tc.tile_pool(name="chunk", bufs=CHUNK_BUFS) as chunkpool):
            n0 = 0
            for F in TILE_FS:
                npts = 128 * F
                xt = io.tile([128, F], F32, tag="x")
                pt = io.tile([128, F], F32, tag="phi")
                nc.sync.dma_start(
                    xt[:], ct_d.ap()[n0:n0 + npts].rearrange("(p f) -> p f", p=128))
                nc.sync.dma_start(
                    pt[:], ph_d.ap()[n0:n0 + npts].rearrange("(p f) -> p f", p=128))

                y32 = sca.tile([128, F], F32, tag="y32")
                nc.scalar.activation(y32[:], xt[:], AFT.Square)
                sh = sca.tile([128, F], F16, tag="sh")
                nc.scalar.activation(sh[:], y32[:], AFT.Sqrt, scale=-1.0, bias=1.0)
                xh = sca.tile([128, F], F16, tag="xh")
                nc.scalar.activation(xh[:], xt[:], AFT.Copy)
                yh = sca.tile([128, F], F16, tag="yh")
                nc.scalar.activation(yh[:], xt[:], AFT.Square)

                # trig: f_m = m*phi/2pi - round(.) packed into one [128, 7F]
                # tile (fp16), then ONE wide Abs + TWO wide Sin activations.
                f7 = sca.tile([128, 7 * F], F16, tag="f7", bufs=1)
                nc.scalar.activation(f7[:, 0:F], pt[:], AFT.Copy, scale=1 / (2 * PI))
                for m in range(2, LMAX + 1):
                    sc = m / (2 * PI)
                    tp = sca.tile([128, F], F32, tag="tp")
                    nc.scalar.activation(tp[:], pt[:], AFT.Identity,
                                         scale=sc, bias=RND_C)
                    ut = sca.tile([128, F], F32, tag="u")
                    nc.scalar.activation(ut[:], tp[:], AFT.Identity,
                                         scale=1.0, bias=-RND_C)
                    nc.vector.scalar_tensor_tensor(
                        f7[:, (m - 1) * F:m * F], pt[:], sc, ut[:],
                        ALU.mult, ALU.subtract)
                z7 = sca.tile([128, 7 * F], F16, tag="z7", bufs=1)
                nc.scalar.activation(z7[:], f7[:], AFT.Abs)
                sin7 = trig.tile([128, 7 * F], F16, tag="sin7")
                nc.scalar.activation(sin7[:], f7[:], AFT.Sin, scale=2 * PI)
                cos7 = trig.tile([128, 7 * F], F16, tag="cos7")
                nc.scalar.activation(cos7[:], z7[:], AFT.Sin,
                                     scale=-2 * PI, bias=PI / 2)
                sint = {m: sin7[:, (m - 1) * F:m * F] for m in range(1, 8)}
                cost = {m: cos7[:, (m - 1) * F:m * F] for m in range(1, 8)}

                # ---- m = 0 chain + columns (chunk0 rows 0..7) ----
                ch0 = chunkpool.tile([128, 8 * F], F16, tag="chunk", name="ch0")
                nc.gpsimd.memset(ch0[:, 0:F], KT[(0, 0)])
                if TS_ENGINE == 'scalar':
                    nc.scalar.activation(ch0[:, F:2 * F], xh[:], AFT.Copy,
                                         scale=float(KT[(1, 0)]))
                else:
                    nc.vector.tensor_scalar(ch0[:, F:2 * F], xh[:],
                                            float(KT[(1, 0)]), None, ALU.mult)
                P = {(1, 0): xh}
                P[(2, 0)] = leg.tile([128, F], F16, tag="pm0_2", name="P2_0")
                nc.vector.tensor_scalar(P[(2, 0)][:], yh[:], float(CREC[(2, 0)]),
                                        -1.0, ALU.mult, ALU.add)
                for l in range(3, LMAX + 1):
                    tmp = tmppool.tile([128, F], F16, tag="tmp", name=f"t0_{l}")
                    (nc.gpsimd if GP_COMPUTE else nc.vector).tensor_mul(tmp[:], xh[:], P[(l - 1, 0)][:])
                    P[(l, 0)] = leg.tile([128, F], F16, tag=f"pm0_{l % 3}",
                                         name=f"P{l}_0")
                    nc.vector.scalar_tensor_tensor(
                        P[(l, 0)][:], tmp[:], float(CREC[(l, 0)]),
                        P[(l - 2, 0)][:], ALU.mult, ALU.subtract)
                for l in range(2, LMAX + 1):
                    nc.scalar.activation(ch0[:, l * F:(l + 1) * F],
                                         P[(l, 0)][:], AFT.Copy,
                                         scale=float(KT[(l, 0)]))
                dst = out_d.ap()[0:8, n0:n0 + npts].rearrange(
                    "g (p f) -> p g f", p=128)
                nc.sync.dma_start(dst, ch0[:, :8 * F].rearrange(
                    "p (g f) -> p g f", g=8))

                # ---- m = 1..7 chains, Phat, wide columns ----
                D = sh
                for m in range(1, LMAX + 1):
                    R = 8 - m
                    if m >= 2:
                        Dn = leg.tile([128, F], F16, tag=f"D{m % 2}", name=f"D{m}")
                        (nc.gpsimd if GP_COMPUTE else nc.vector).tensor_mul(Dn[:], sh[:], D[:])
                        D = Dn
                    ph = phatpool.tile([128, R * F], F16, tag="phat",
                                       name=f"phat{m}")
                    if TS_ENGINE == 'scalar':
                        nc.scalar.activation(ph[:, 0:F], D[:], AFT.Copy,
                                             scale=float(KT[(m, m)]))
                    else:
                        nc.vector.tensor_scalar(ph[:, 0:F], D[:],
                                                float(KT[(m, m)]), None, ALU.mult)
                    if m < LMAX:
                        T = leg.tile([128, F], F16, tag="T", name=f"T{m}")
                        (nc.gpsimd if GP_COMPUTE else nc.vector).tensor_mul(T[:], xh[:], D[:])
                        nc.scalar.activation(ph[:, F:2 * F], T[:], AFT.Copy,
                                             scale=float(KT[(m + 1, m)]))
                        prev2, prev = D, T
                        for l in range(m + 2, LMAX + 1):
                            j = l - m
                            tmp = tmppool.tile([128, F], F16, tag="tmp",
                                               name=f"t{m}_{l}")
                            eng = nc.gpsimd if (GP_COMPUTE and (l + m) % 2 == 0) else nc.vector
                            eng.tensor_mul(tmp[:], xh[:], prev[:])
                            Pl = leg.tile([128, F], F16, tag=f"p{m % 2}_{l % 3}",
                                          name=f"P{l}_{m}")
                            nc.vector.scalar_tensor_tensor(
                                Pl[:], tmp[:], float(CREC[(l, m)]), prev2[:],
                                ALU.mult, ALU.subtract)
                            if j % 2 == 0 and TS_ENGINE != 'scalar':
                                nc.vector.tensor_scalar(
                                    ph[:, j * F:(j + 1) * F], Pl[:],
                                    float(KT[(l, m)]), None, ALU.mult)
                            elif j % 2 == 0 or True:
                                nc.scalar.activation(
                                    ph[:, j * F:(j + 1) * F], Pl[:], AFT.Copy,
                                    scale=float(KT[(l, m)]))
                            prev2, prev = prev, Pl
                    chm = chunkpool.tile([128, 2 * R * F], F16, tag="chunk",
                                         name=f"ch{m}")
                    ph3 = ph[:, :R * F].rearrange("p (j f) -> p j f", j=R)
                    nc.vector.tensor_tensor(
                        chm[:, :R * F].rearrange("p (j f) -> p j f", j=R),
                        ph3, _rep(sint[m], R), ALU.mult)
                    nc.vector.tensor_tensor(
                        chm[:, R * F:2 * R * F].rearrange("p (j f) -> p j f", j=R),
                        ph3, _rep(cost[m], R), ALU.mult)
                    r0 = ROWS_SIN[m]
                    dst = out_d.ap()[r0:r0 + 2 * R, n0:n0 + npts].rearrange(
                        "g (p f) -> p g f", p=128)
                    nc.sync.dma_start(dst, chm[:, :2 * R * F].rearrange(
                        "p (g f) -> p g f", g=2 * R))
                n0 += npts

    nc.compile()
    return nc


_NC = None


def _forward(ct_np, ph_np, trace=False):
    global _NC
    if _NC is None:
        _NC = _build()
    ct_pad = np.zeros(N_PAD, np.float32)
    ph_pad = np.zeros(N_PAD, np.float32)
    n = min(len(ct_np), N_PAD)
    ct_pad[:n] = ct_np[:n]
    ph_pad[:n] = ph_np[:n]
    in_maps = [{"ct": ct_pad[i * NSH:(i + 1) * NSH],
                "phi": ph_pad[i * NSH:(i + 1) * NSH]} for i in range(NCORES)]
    res = run_bass_kernel_spmd(_NC, in_maps, list(range(NCORES)), trace=trace)
    full = np.empty((NCOL, N_PAD), np.float32)
    for i, r in enumerate(res.results):
        o = np.asarray(r["out"])           # [64, NSH] fp16, m-major rows
        full[:, i * NSH:(i + 1) * NSH] = o[PERM].astype(np.float32)
    return full, res


def kernel(cos_theta, phi, l_max):
    assert int(l_max) == LMAX
    ct_np = np.asarray(cos_theta, dtype=np.float32).reshape(-1)
    ph_np = np.asarray(phi, dtype=np.float32).reshape(-1)
    n = ct_np.shape[0]
    full, _ = _forward(ct_np, ph_np)
    return tuple(full[l * l:(l + 1) * (l + 1), :n].T for l in range(LMAX + 1))


# revision 15
# speedup vs baseline: 1.0282x; 1.0282x over previous
"""Real spherical harmonics Y_l^m (l=0..7) over N=2M points on 8 TRN2 NeuronCores.

Design (fp16 compute, ~2x DVE rate, ~1.5e-3 rel err vs f32 reference):
  - Data-parallel over points: each core gets NSH=250112 points (global input
    padded from 2,000,000 to 2,000,896).
  - Per core, tiles of [128 partitions, F free]; point id = tile_off + p*F + f.
  - Output stored column-major in DRAM ([64 rows, NSH] fp16, m-major row
    order) so every engine write and DMA is contiguous; host converts to f32
    and regroups per-l.
  - Scaled Legendre recurrences: Pt(l,m) = P(l,m)/gamma(l,m) with gamma chosen
    so  Pt(m,m)=s*Pt(m-1,m-1), Pt(m+1,m)=x*Pt(m,m),
        Pt(l,m)=c(l,m)*x*Pt(l-1,m) - Pt(l-2,m)
  - sin/cos(m*phi) on ScalarE: fp32 round-to-nearest trick (+1.5*2^23) for
    range reduction, then Sin activations (table valid on [-3.4, 3.4]).
  - Columns: Phat(l,m) = K(l,m)*Pt(l,m) (tensor_scalar, 4x fp16), written
    contiguously per m; then ONE wide tensor_tensor per (m, sin/cos) using a
    stride-0 broadcast AP of the trig tile (2x fp16).
  - Work split across VectorE / GpSimd / ScalarE; DMA via HWDGE (nc.sync).
"""
import sys, math, dataclasses

if '/opt/trn_rl_repo' not in sys.path:
    sys.path.insert(0, '/opt/trn_rl_repo')

import numpy as np
import concourse.bass as bass
import concourse.tile as tile
from concourse import bacc, mybir
from concourse.bass_utils import run_bass_kernel_spmd

AFT = mybir.ActivationFunctionType
ALU = mybir.AluOpType
F32 = mybir.dt.float32
F16 = mybir.dt.float16

N_FULL = 2_000_000
NCORES = 8
LMAX = 7
NCOL = 64
NSH = 250_112                           # 128 * 1954
N_PAD = NSH * NCORES
TILE_FS = [652, 652, 650]
PI = math.pi
RND_C = float(3 * 2 ** 22)
GP_COMPUTE = False                      # GpSimd contends with VectorE SBUF ports
TS_ENGINE = 'scalar'                    # Phat scaling on ScalarE (Copy w/ scale)
CHUNK_BUFS = 3
PHAT_BUFS = 2
TRIG_BUFS = 2

# DRAM row order (m-major): rows 0..7 = (l,0) l=0..7; then per m: sin rows
# (l=m..7), cos rows (l=m..7).
ROW0 = {}
_r = 8
ROWS_SIN, ROWS_COS = {}, {}
for _m in range(1, 8):
    ROWS_SIN[_m] = _r; _r += 8 - _m
    ROWS_COS[_m] = _r; _r += 8 - _m
assert _r == 64

# PERM[c_lmajor] = dram row for global column c = l*l + l + m
PERM = np.zeros(64, np.int64)
for _l in range(8):
    for _mm in range(-_l, _l + 1):
        _c = _l * _l + _l + _mm
        if _mm == 0:
            PERM[_c] = _l
        elif _mm < 0:
            PERM[_c] = ROWS_SIN[-_mm] + (_l - (-_mm))
        else:
            PERM[_c] = ROWS_COS[_mm] + (_l - _mm)


def _constants():
    g = {(0, 0): 1.0}
    c = {}
    for m in range(1, LMAX + 1):
        g[(m, m)] = g[(m - 1, m - 1)] * (-(2 * m - 1))
    for m in range(0, LMAX):
        g[(m + 1, m)] = g[(m, m)] * (2 * m + 1)
    for m in range(0, LMAX + 1):
        for l in range(m + 2, LMAX + 1):
            g[(l, m)] = (l + m - 1) * g[(l - 2, m)] / (l - m)
            c[(l, m)] = (2 * l - 1) * g[(l - 1, m)] / ((l - m) * g[(l, m)])
    kt = {}
    for l in range(LMAX + 1):
        for m in range(0, l + 1):
            kn = math.sqrt((2 * l + 1) / (4 * math.pi)
                           * math.factorial(l - m) / math.factorial(l + m))
            kt[(l, m)] = kn * g[(l, m)] * (math.sqrt(2.0) if m > 0 else 1.0)
    return c, kt


CREC, KT = _constants()


def _rep(ap, r):
    """Broadcast a [128, F] AP to [128, r, F] via a stride-0 middle dim."""
    return dataclasses.replace(ap, ap=[ap.ap[0], [0, r], ap.ap[1]])


def _build():
    nc = bacc.Bacc("TRN2", target_bir_lowering=False, debug=False,
                   num_devices=NCORES)

    def reg_const(value):
        t = nc.alloc_sbuf_tensor(f"cst{value}", [128, 1], F32)
        nc.gpsimd.memset(t.ap(), value)
        nc.const_aps.aps[(F32, value)] = t.ap()

    reg_const(PI / 2)
    reg_const(RND_C)
    reg_const(-RND_C)
    nc.all_engine_barrier()

    ct_d = nc.dram_tensor("ct", [NSH], F32, kind="ExternalInput")
    ph_d = nc.dram_tensor("phi", [NSH], F32, kind="ExternalInput")
    out_d = nc.dram_tensor("out", [NCOL, NSH], F16, kind="ExternalOutput")

    with tile.TileContext(nc) as tc:
        with (tc.tile_pool(name="io", bufs=2) as io,
              tc.tile_pool(name="sca", bufs=2) as sca,
              tc.tile_pool(name="trig", bufs=TRIG_BUFS) as trig,
              tc.tile_pool(name="leg", bufs=2) as leg,
              tc.tile_pool(name="tmp", bufs=4) as tmppool,
              tc.tile_pool(name="phat", bufs=PHAT_BUFS) as phatpool,
              tc.tile_pool(name="chunk", bufs=CHUNK_BUFS) as chunkpool):
            n0 = 0
            for F in TILE_FS:
                npts = 128 * F
                xt = io.tile([128, F], F32, tag="x")
                pt = io.tile([128, F], F32, tag="phi")
                nc.sync.dma_start(
                    xt[:], ct_d.ap()[n0:n0 + npts].rearrange("(p f) -> p f", p=128))
                nc.sync.dma_start(
                    pt[:], ph_d.ap()[n0:n0 + npts].rearrange("(p f) -> p f", p=128))

                y32 = sca.tile([128, F], F32, tag="y32")
                nc.scalar.activation(y32[:], xt[:], AFT.Square)
                sh = sca.tile([128, F], F16, tag="sh")
                nc.scalar.activation(sh[:], y32[:], AFT.Sqrt, scale=-1.0, bias=1.0)
                xh = sca.tile([128, F], F16, tag="xh")
                nc.scalar.activation(xh[:], xt[:], AFT.Copy)
                yh = sca.tile([128, F], F16, tag="yh")
                nc.scalar.activation(yh[:], xt[:], AFT.Square)

                # trig: f_m = m*phi/2pi - round(.) packed into one [128, 7F]
                # tile (fp16), then ONE wide Abs + TWO wide Sin activations.
                f7 = sca.tile([128, 7 * F], F16, tag="f7", bufs=1)
                nc.scalar.activation(f7[:, 0:F], pt[:], AFT.Copy, scale=1 / (2 * PI))
                for m in range(2, LMAX + 1):
                    sc = m / (2 * PI)
                    tp = sca.tile([128, F], F32, tag="tp")
                    nc.scalar.activation(tp[:], pt[:], AFT.Identity,
                                         scale=sc, bias=RND_C)
                    ut = sca.tile([128, F], F32, tag="u")
                    nc.scalar.activation(ut[:], tp[:], AFT.Identity,
                                         scale=1.0, bias=-RND_C)
                    nc.vector.scalar_tensor_tensor(
                        f7[:, (m - 1) * F:m * F], pt[:], sc, ut[:],
                        ALU.mult, ALU.subtract)
                z7 = sca.tile([128, 7 * F], F16, tag="z7", bufs=1)
                nc.scalar.activation(z7[:], f7[:], AFT.Abs)
                sin7 = trig.tile([128, 7 * F], F16, tag="sin7")
                nc.scalar.activation(sin7[:], f7[:], AFT.Sin, scale=2 * PI)
                cos7 = trig.tile([128, 7 * F], F16, tag="cos7")
                nc.scalar.activation(cos7[:], z7[:], AFT.Sin,
                                     scale=-2 * PI, bias=PI / 2)
                sint = {m: sin7[:, (m - 1) * F:m * F] for m in range(1, 8)}
                cost = {m: cos7[:, (m - 1) * F:m * F] for m in range(1, 8)}

                # ---- m = 0 chain + columns (chunk0 rows 0..7) ----
                ch0 = chunkpool.tile([128, 8 * F], F16, tag="chunk", name="ch0")
                nc.gpsimd.memset(ch0[:, 0:F], KT[(0, 0)])
                if TS_ENGINE == 'scalar':
                    nc.scalar.activation(ch0[:, F:2 * F], xh[:], AFT.Copy,
                                         scale=float(KT[(1, 0)]))
                else:
                    nc.vector.tensor_scalar(ch0[:, F:2 * F], xh[:],
                                            float(KT[(1, 0)]), None, ALU.mult)
                P = {(1, 0): xh}
                P[(2, 0)] = leg.tile([128, F], F16, tag="pm0_2", name="P2_0")
                nc.vector.tensor_scalar(P[(2, 0)][:], yh[:], float(CREC[(2, 0)]),
                                        -1.0, ALU.mult, ALU.add)
                for l in range(3, LMAX + 1):
                    cx = tmppool.tile([128, F], F16, tag="cx", name=f"cx0_{l}")
                    nc.vector.tensor_scalar(cx[:], xh[:], float(CREC[(l, 0)]),
                                            None, ALU.mult)
                    tmp = tmppool.tile([128, F], F16, tag="tmp", name=f"t0_{l}")
                    nc.vector.tensor_mul(tmp[:], cx[:], P[(l - 1, 0)][:])
                    P[(l, 0)] = leg.tile([128, F], F16, tag=f"pm0_{l % 3}",
                                         name=f"P{l}_0")
                    nc.vector.tensor_sub(P[(l, 0)][:], tmp[:], P[(l - 2, 0)][:])
                for l in range(2, LMAX + 1):
                    nc.scalar.activation(ch0[:, l * F:(l + 1) * F],
                                         P[(l, 0)][:], AFT.Copy,
                                         scale=float(KT[(l, 0)]))
                dst = out_d.ap()[0:8, n0:n0 + npts].rearrange(
                    "g (p f) -> p g f", p=128)
                nc.sync.dma_start(dst, ch0[:, :8 * F].rearrange(
                    "p (g f) -> p g f", g=8))

                # ---- m = 1..7 chains, Phat, wide columns ----
                D = sh
                for m in range(1, LMAX + 1):
                    R = 8 - m
                    if m >= 2:
                        Dn = leg.tile([128, F], F16, tag=f"D{m % 2}", name=f"D{m}")
                        (nc.gpsimd if GP_COMPUTE else nc.vector).tensor_mul(Dn[:], sh[:], D[:])
                        D = Dn
                    ph = phatpool.tile([128, R * F], F16, tag="phat",
                                       name=f"phat{m}")
                    if TS_ENGINE == 'scalar':
                        nc.scalar.activation(ph[:, 0:F], D[:], AFT.Copy,
                                             scale=float(KT[(m, m)]))
                    else:
                        nc.vector.tensor_scalar(ph[:, 0:F], D[:],
                                                float(KT[(m, m)]), None, ALU.mult)
                    if m < LMAX:
                        T = leg.tile([128, F], F16, tag="T", name=f"T{m}")
                        (nc.gpsimd if GP_COMPUTE else nc.vector).tensor_mul(T[:], xh[:], D[:])
                        nc.scalar.activation(ph[:, F:2 * F], T[:], AFT.Copy,
                                             scale=float(KT[(m + 1, m)]))
                        prev2, prev = D, T
                        for l in range(m + 2, LMAX + 1):
                            j = l - m
                            cx = tmppool.tile([128, F], F16, tag="cx",
                                              name=f"cx{m}_{l}")
                            nc.vector.tensor_scalar(cx[:], xh[:],
                                                    float(CREC[(l, m)]),
                                                    None, ALU.mult)
                            tmp = tmppool.tile([128, F], F16, tag="tmp",
                                               name=f"t{m}_{l}")
                            nc.vector.tensor_mul(tmp[:], cx[:], prev[:])
                            Pl = leg.tile([128, F], F16, tag=f"p{m % 2}_{l % 3}",
                                          name=f"P{l}_{m}")
                            nc.vector.tensor_sub(Pl[:], tmp[:], prev2[:])
                            if j % 2 == 0 and TS_ENGINE != 'scalar':
                                nc.vector.tensor_scalar(
                                    ph[:, j * F:(j + 1) * F], Pl[:],
                                    float(KT[(l, m)]), None, ALU.mult)
                            elif j % 2 == 0 or True:
                                nc.scalar.activation(
                                    ph[:, j * F:(j + 1) * F], Pl[:], AFT.Copy,
                                    scale=float(KT[(l, m)]))
                            prev2, prev = prev, Pl
                    chm = chunkpool.tile([128, 2 * R * F], F16, tag="chunk",
                                         name=f"ch{m}")
                    ph3 = ph[:, :R * F].rearrange("p (j f) -> p j f", j=R)
                    nc.vector.tensor_tensor(
                        chm[:, :R * F].rearrange("p (j f) -> p j f", j=R),
                        ph3, _rep(sint[m], R), ALU.mult)
                    nc.vector.tensor_tensor(
                        chm[:, R * F:2 * R * F].rearrange("p (j f) -> p j f", j=R),
                        ph3, _rep(cost[m], R), ALU.mult)
                    r0 = ROWS_SIN[m]
                    dst = out_d.ap()[r0:r0 + 2 * R, n0:n0 + npts].rearrange(
                        "g (p f) -> p g f", p=128)
                    nc.sync.dma_start(dst, chm[:, :2 * R * F].rearrange(
                        "p (g f) -> p g f", g=2 * R))
                n0 += npts

    nc.compile()
    return nc


_NC = None


def _forward(ct_np, ph_np, trace=False):
    global _NC
    if _NC is None:
        _NC = _build()
    ct_pad = np.zeros(N_PAD, np.float32)
    ph_pad = np.zeros(N_PAD, np.float32)
    n = min(len(ct_np), N_PAD)
    ct_pad[:n] = ct_np[:n]
    ph_pad[:n] = ph_np[:n]
    in_maps = [{"ct": ct_pad[i * NSH:(i + 1) * NSH],
                "phi": ph_pad[i * NSH:(i + 1) * NSH]} for i in range(NCORES)]
    res = run_bass_kernel_spmd(_NC, in_maps, list(range(NCORES)), trace=trace)
    full = np.empty((NCOL, N_PAD), np.float32)
    for i, r in enumerate(res.results):
        o = np.asarray(r["out"])           # [64, NSH] fp16, m-major rows
        full[:, i * NSH:(i + 1) * NSH] = o[PERM].astype(np.float32)
    return full, res


def kernel(cos_theta, phi, l_max):
    assert int(l_max) == LMAX
    ct_np = np.asarray(cos_theta, dtype=np.float32).reshape(-1)
    ph_np = np.asarray(phi, dtype=np.float32).reshape(-1)
    n = ct_np.shape[0]
    full, _ = _forward(ct_np, ph_np)
    return tuple(full[l * l:(l + 1) * (l + 1), :n].T for l in range(LMAX + 1))


# revision 18
# speedup vs baseline: 1.0418x; 1.0132x over previous
"""Real spherical harmonics Y_l^m (l=0..7) over N=2M points on 8 TRN2 NeuronCores.

Design (fp16 compute, ~2x DVE rate, ~1.5e-3 rel err vs f32 reference):
  - Data-parallel over points: each core gets NSH=250112 points (global input
    padded from 2,000,000 to 2,000,896).
  - Per core, tiles of [128 partitions, F free]; point id = tile_off + p*F + f.
  - Output stored column-major in DRAM ([64 rows, NSH] fp16, m-major row
    order) so every engine write and DMA is contiguous; host converts to f32
    and regroups per-l.
  - Scaled Legendre recurrences: Pt(l,m) = P(l,m)/gamma(l,m) with gamma chosen
    so  Pt(m,m)=s*Pt(m-1,m-1), Pt(m+1,m)=x*Pt(m,m),
        Pt(l,m)=c(l,m)*x*Pt(l-1,m) - Pt(l-2,m)
  - sin/cos(m*phi) on ScalarE: fp32 round-to-nearest trick (+1.5*2^23) for
    range reduction, then Sin activations (table valid on [-3.4, 3.4]).
  - Columns: Phat(l,m) = K(l,m)*Pt(l,m) (tensor_scalar, 4x fp16), written
    contiguously per m; then ONE wide tensor_tensor per (m, sin/cos) using a
    stride-0 broadcast AP of the trig tile (2x fp16).
  - Work split across VectorE / GpSimd / ScalarE; DMA via HWDGE (nc.sync).
"""
import sys, math, dataclasses

if '/opt/trn_rl_repo' not in sys.path:
    sys.path.insert(0, '/opt/trn_rl_repo')

import numpy as np
import concourse.bass as bass
import concourse.tile as tile
from concourse import bacc, mybir
from concourse.bass_utils import run_bass_kernel_spmd

AFT = mybir.ActivationFunctionType
ALU = mybir.AluOpType
F32 = mybir.dt.float32
F16 = mybir.dt.float16

N_FULL = 2_000_000
NCORES = 8
LMAX = 7
NCOL = 64
NSH = 250_112                           # 128 * 1954
N_PAD = NSH * NCORES
TILE_FS = [652, 652, 650]
PI = math.pi
RND_C = float(3 * 2 ** 22)
GP_COMPUTE = False                      # GpSimd contends with VectorE SBUF ports
TS_ENGINE = 'scalar'                    # Phat scaling on ScalarE (Copy w/ scale)
CHUNK_BUFS = 3
PHAT_BUFS = 2
TRIG_BUFS = 2

# DRAM row order (m-major): rows 0..7 = (l,0) l=0..7; then per m: sin rows
# (l=m..7), cos rows (l=m..7).
ROW0 = {}
_r = 8
ROWS_SIN, ROWS_COS = {}, {}
for _m in range(1, 8):
    ROWS_SIN[_m] = _r; _r += 8 - _m
    ROWS_COS[_m] = _r; _r += 8 - _m
assert _r == 64

# PERM[c_lmajor] = dram row for global column c = l*l + l + m
PERM = np.zeros(64, np.int64)
for _l in range(8):
    for _mm in range(-_l, _l + 1):
        _c = _l * _l + _l + _mm
        if _mm == 0:
            PERM[_c] = _l
        elif _mm < 0:
            PERM[_c] = ROWS_SIN[-_mm] + (_l - (-_mm))
        else:
            PERM[_c] = ROWS_COS[_mm] + (_l - _mm)


def _constants():
    g = {(0, 0): 1.0}
    c = {}
    for m in range(1, LMAX + 1):
        g[(m, m)] = g[(m - 1, m - 1)] * (-(2 * m - 1))
    for m in range(0, LMAX):
        g[(m + 1, m)] = g[(m, m)] * (2 * m + 1)
    for m in range(0, LMAX + 1):
        for l in range(m + 2, LMAX + 1):
            g[(l, m)] = (l + m - 1) * g[(l - 2, m)] / (l - m)
            c[(l, m)] = (2 * l - 1) * g[(l - 1, m)] / ((l - m) * g[(l, m)])
    kt = {}
    for l in range(LMAX + 1):
        for m in range(0, l + 1):
            kn = math.sqrt((2 * l + 1) / (4 * math.pi)
                           * math.factorial(l - m) / math.factorial(l + m))
            kt[(l, m)] = kn * g[(l, m)] * (math.sqrt(2.0) if m > 0 else 1.0)
    return c, kt


CREC, KT = _constants()


def _rep(ap, r):
    """Broadcast a [128, F] AP to [128, r, F] via a stride-0 middle dim."""
    return dataclasses.replace(ap, ap=[ap.ap[0], [0, r], ap.ap[1]])


def _build():
    nc = bacc.Bacc("TRN2", target_bir_lowering=False, debug=False,
                   num_devices=NCORES)

    def reg_const(value):
        t = nc.alloc_sbuf_tensor(f"cst{value}", [128, 1], F32)
        nc.gpsimd.memset(t.ap(), value)
        nc.const_aps.aps[(F32, value)] = t.ap()

    reg_const(PI / 2)
    reg_const(RND_C)
    reg_const(-RND_C)
    nc.all_engine_barrier()

    ct_d = nc.dram_tensor("ct", [NSH], F32, kind="ExternalInput")
    ph_d = nc.dram_tensor("phi", [NSH], F32, kind="ExternalInput")
    out_d = nc.dram_tensor("out", [NCOL, NSH], F16, kind="ExternalOutput")

    with tile.TileContext(nc) as tc:
        with (tc.tile_pool(name="io", bufs=2) as io,
              tc.tile_pool(name="sca", bufs=2) as sca,
              tc.tile_pool(name="trig", bufs=TRIG_BUFS) as trig,
              tc.tile_pool(name="leg", bufs=2) as leg,
              tc.tile_pool(name="tmp", bufs=4) as tmppool,
              tc.tile_pool(name="phat", bufs=PHAT_BUFS) as phatpool,
              tc.tile_pool(name="chunk", bufs=CHUNK_BUFS) as chunkpool):
            n0 = 0
            for F in TILE_FS:
                npts = 128 * F
                xt = io.tile([128, F], F32, tag="x")
                pt = io.tile([128, F], F32, tag="phi")
                nc.sync.dma_start(
                    xt[:], ct_d.ap()[n0:n0 + npts].rearrange("(p f) -> p f", p=128))
                nc.sync.dma_start(
                    pt[:], ph_d.ap()[n0:n0 + npts].rearrange("(p f) -> p f", p=128))

                y32 = sca.tile([128, F], F32, tag="y32")
                nc.scalar.activation(y32[:], xt[:], AFT.Square)
                sh = sca.tile([128, F], F16, tag="sh")
                nc.scalar.activation(sh[:], y32[:], AFT.Sqrt, scale=-1.0, bias=1.0)
                xh = sca.tile([128, F], F16, tag="xh")
                nc.scalar.activation(xh[:], xt[:], AFT.Copy)
                yh = sca.tile([128, F], F16, tag="yh")
                nc.scalar.activation(yh[:], xt[:], AFT.Square)

                # trig: f_m = m*phi/2pi - round(.) packed into one [128, 7F]
                # tile (fp16), then ONE wide Abs + TWO wide Sin activations.
                f7 = sca.tile([128, 7 * F], F16, tag="f7", bufs=1)
                nc.scalar.activation(f7[:, 0:F], pt[:], AFT.Copy, scale=1 / (2 * PI))
                for m in range(2, LMAX + 1):
                    sc = m / (2 * PI)
                    tp = sca.tile([128, F], F32, tag="tp")
                    nc.scalar.activation(tp[:], pt[:], AFT.Identity,
                                         scale=sc, bias=RND_C)
                    ut = sca.tile([128, F], F32, tag="u")
                    nc.scalar.activation(ut[:], tp[:], AFT.Identity,
                                         scale=1.0, bias=-RND_C)
                    nc.vector.scalar_tensor_tensor(
                        f7[:, (m - 1) * F:m * F], pt[:], sc, ut[:],
                        ALU.mult, ALU.subtract)
                z7 = sca.tile([128, 7 * F], F16, tag="z7", bufs=1)
                nc.scalar.activation(z7[:], f7[:], AFT.Abs)
                sin7 = trig.tile([128, 7 * F], F16, tag="sin7")
                nc.scalar.activation(sin7[:], f7[:], AFT.Sin, scale=2 * PI)
                cos7 = trig.tile([128, 7 * F], F16, tag="cos7")
                nc.scalar.activation(cos7[:], z7[:], AFT.Sin,
                                     scale=-2 * PI, bias=PI / 2)
                sint = {m: sin7[:, (m - 1) * F:m * F] for m in range(1, 8)}
                cost = {m: cos7[:, (m - 1) * F:m * F] for m in range(1, 8)}

                # ---- m = 0 chain + columns (chunk0 rows 0..7) ----
                ch0 = chunkpool.tile([128, 8 * F], F16, tag="chunk", name="ch0")
                nc.gpsimd.memset(ch0[:, 0:F], KT[(0, 0)])
                if TS_ENGINE == 'scalar':
                    nc.scalar.activation(ch0[:, F:2 * F], xh[:], AFT.Copy,
                                         scale=float(KT[(1, 0)]))
                else:
                    nc.vector.tensor_scalar(ch0[:, F:2 * F], xh[:],
                                            float(KT[(1, 0)]), None, ALU.mult)
                P = {(1, 0): xh}
                P[(2, 0)] = leg.tile([128, F], F16, tag="pm0_2", name="P2_0")
                nc.vector.tensor_scalar(P[(2, 0)][:], yh[:], float(CREC[(2, 0)]),
                                        -1.0, ALU.mult, ALU.add)
                for l in range(3, LMAX + 1):
                    cx = tmppool.tile([128, F], F16, tag="cx", name=f"cx0_{l}")
                    nc.vector.tensor_scalar(cx[:], xh[:], float(CREC[(l, 0)]),
                                            None, ALU.mult)
                    tmp = tmppool.tile([128, F], F16, tag="tmp", name=f"t0_{l}")
                    nc.vector.tensor_mul(tmp[:], cx[:], P[(l - 1, 0)][:])
                    P[(l, 0)] = leg.tile([128, F], F16, tag=f"pm0_{l % 3}",
                                         name=f"P{l}_0")
                    nc.vector.tensor_sub(P[(l, 0)][:], tmp[:], P[(l - 2, 0)][:])
                for l in range(2, LMAX + 1):
                    nc.scalar.activation(ch0[:, l * F:(l + 1) * F],
                                         P[(l, 0)][:], AFT.Copy,
                                         scale=float(KT[(l, 0)]))
                dst = out_d.ap()[0:8, n0:n0 + npts].rearrange(
                    "g (p f) -> p g f", p=128)
                nc.sync.dma_start(dst, ch0[:, :8 * F].rearrange(
                    "p (g f) -> p g f", g=8))

                # ---- m = 1..7 chains, Phat, wide columns ----
                D = sh
                for m in range(1, LMAX + 1):
                    R = 8 - m
                    if m >= 2:
                        Dn = leg.tile([128, F], F16, tag=f"D{m % 2}", name=f"D{m}")
                        (nc.gpsimd if GP_COMPUTE else nc.vector).tensor_mul(Dn[:], sh[:], D[:])
                        D = Dn
                    ph = phatpool.tile([128, R * F], F16, tag="phat",
                                       name=f"phat{m}")
                    if TS_ENGINE == 'scalar':
                        nc.scalar.activation(ph[:, 0:F], D[:], AFT.Copy,
                                             scale=float(KT[(m, m)]))
                    else:
                        nc.vector.tensor_scalar(ph[:, 0:F], D[:],
                                                float(KT[(m, m)]), None, ALU.mult)
                    if m < LMAX:
                        T = leg.tile([128, F], F16, tag="T", name=f"T{m}")
                        (nc.gpsimd if GP_COMPUTE else nc.vector).tensor_mul(T[:], xh[:], D[:])
                        nc.scalar.activation(ph[:, F:2 * F], T[:], AFT.Copy,
                                             scale=float(KT[(m + 1, m)]))
                        prev2, prev = D, T
                        for l in range(m + 2, LMAX + 1):
                            j = l - m
                            cx = tmppool.tile([128, F], F16, tag="cx",
                                              name=f"cx{m}_{l}")
                            nc.vector.tensor_scalar(cx[:], xh[:],
                                                    float(CREC[(l, m)]),
                                                    None, ALU.mult)
                            tmp = tmppool.tile([128, F], F16, tag="tmp",
                                               name=f"t{m}_{l}")
                            nc.vector.tensor_mul(tmp[:], cx[:], prev[:])
                            Pl = leg.tile([128, F], F16, tag=f"p{m % 2}_{l % 3}",
                                          name=f"P{l}_{m}")
                            nc.vector.tensor_sub(Pl[:], tmp[:], prev2[:])
                            if j % 2 == 0 and TS_ENGINE != 'scalar':
                                nc.vector.tensor_scalar(
                                    ph[:, j * F:(j + 1) * F], Pl[:],
                                    float(KT[(l, m)]), None, ALU.mult)
                            elif j % 2 == 0 or True:
                                nc.scalar.activation(
                                    ph[:, j * F:(j + 1) * F], Pl[:], AFT.Copy,
                                    scale=float(KT[(l, m)]))
                            prev2, prev = prev, Pl
                    chm = chunkpool.tile([128, 2 * R * F], F16, tag="chunk",
                                         name=f"ch{m}")
                    ph3 = ph[:, :R * F].rearrange("p (j f) -> p j f", j=R)
                    nc.vector.tensor_tensor(
                        chm[:, :R * F].rearrange("p (j f) -> p j f", j=R),
                        ph3, _rep(sint[m], R), ALU.mult)
                    nc.vector.tensor_tensor(
                        chm[:, R * F:2 * R * F].rearrange("p (j f) -> p j f", j=R),
                        ph3, _rep(cost[m], R), ALU.mult)
                    r0 = ROWS_SIN[m]
                    dst = out_d.ap()[r0:r0 + 2 * R, n0:n0 + npts].rearrange(
                        "g (p f) -> p g f", p=128)
                    nc.sync.dma_start(dst, chm[:, :2 * R * F].rearrange(
                        "p (g f) -> p g f", g=2 * R))
                n0 += npts

    nc.compile()
    return nc


_NC = None


def _forward(ct_np, ph_np, trace=False):
    global _NC
    if _NC is None:
        _NC = _build()
    ct_pad = np.zeros(N_PAD, np.float32)
    ph_pad = np.zeros(N_PAD, np.float32)
    n = min(len(ct_np), N_PAD)
    ct_pad[:n] = ct_np[:n]
    ph_pad[:n] = ph_np[:n]
    in_maps = [{"ct": ct_pad[i * NSH:(i + 1) * NSH],
                "phi": ph_pad[i * NSH:(i + 1) * NSH]} for i in range(NCORES)]
    res = run_bass_kernel_spmd(_NC, in_maps, list(range(NCORES)), trace=trace)
    full = np.empty((NCOL, N_PAD), np.float32)
    for i, r in enumerate(res.results):
        o = np.asarray(r["out"])           # [64, NSH] fp16, m-major rows
        full[:, i * NSH:(i + 1) * NSH] = o[PERM].astype(np.float32)
    return full, res


def kernel(cos_theta, phi, l_max):
    assert int(l_max) == LMAX
    ct_np = np.asarray(cos_theta, dtype=np.float32).reshape(-1)
    ph_np = np.asarray(phi, dtype=np.float32).reshape(-1)
    n = ct_np.shape[0]
    full, _ = _forward(ct_np, ph_np)
    return tuple(full[l * l:(l + 1) * (l + 1), :n].T for l in range(LMAX + 1))
